# revision 1
# baseline (speedup 1.0000x reference)
"""Trainium2 kernel for nn_HSCR_67396626809127 (gnn_message_passing).

The reference network (fc1/fc2 -> 24-step KTD kinematic-tree recurrence ->
cam/pose/shape heads) contains no nonlinearity (dropout is identity in eval
mode), so the whole module is one affine map:

    out[157] = W @ [x(256) | init_pose(144) | init_shape(10) | init_cam(3)] + b

W [157,413] / b [157] are composed on host in float64 from the small weight
tensors (<5MB total), with the bias folded in as a constant-ones feature row
(K = 414).  The device then runs a single data-parallel matmul over the
B*T = 32768 tokens: each of the 8 cores handles 4096 tokens, reading
feature-major activation tiles (transposed on host) and writing a
feature-major output tile that the host transposes back.
"""

import numpy as np

ANCESTOR_INDEX = [[], [0], [0], [0], [0, 1], [0, 2], [0, 3], [0, 1, 4],
                  [0, 2, 5], [0, 3, 6], [0, 1, 4, 7], [0, 2, 5, 8],
                  [0, 3, 6, 9], [0, 3, 6, 9], [0, 3, 6, 9], [0, 3, 6, 9, 12],
                  [0, 3, 6, 9, 13], [0, 3, 6, 9, 14], [0, 3, 6, 9, 13, 16],
                  [0, 3, 6, 9, 14, 17], [0, 3, 6, 9, 13, 16, 18],
                  [0, 3, 6, 9, 14, 17, 19], [0, 3, 6, 9, 13, 16, 18, 20],
                  [0, 3, 6, 9, 14, 17, 19, 21]]
HID = 1024
NCORES = 8
B, T = 2048, 16
NTOK = B * T                 # 32768
TPC = NTOK // NCORES         # 4096 tokens per core
NOUT = 157                   # [cam 3 | pose 144 | shape 10]
KV = 414                     # 413 input features + ones row (bias)
TW = 1024                    # tokens per SBUF tile
NT = TPC // TW               # 4 tiles per core
MCH = [(0, 128), (128, 29)]  # output-feature chunks (psum partition dim)

_PROG = {}


def _compose_affine(fc1_w, fc1_b, fc2_w, fc2_b, decshape_w, decshape_b,
                    deccam_w, deccam_b, ktd_w, ktd_b):
    """Fold the whole network into out = v @ W.T + b, v = [x|pose|shape|cam]."""
    f8 = np.float64
    fc1_w, fc1_b = fc1_w.astype(f8), fc1_b.astype(f8)
    fc2_w, fc2_b = fc2_w.astype(f8), fc2_b.astype(f8)
    decshape_w, decshape_b = decshape_w.astype(f8), decshape_b.astype(f8)
    deccam_w, deccam_b = deccam_w.astype(f8), deccam_b.astype(f8)
    ktd_w, ktd_b = ktd_w.astype(f8), ktd_b.astype(f8)

    F1x, F1s = fc1_w[:, :256], fc1_w[:, 256:266]
    F2x, F2p = fc2_w[:, :256], fc2_w[:, 256:400]

    # KTD recurrence -> pose_out = G @ xc_pose + H @ init_pose + c
    G = np.zeros((24, 6, HID)); H = np.zeros((24, 6, 144)); c = np.zeros((24, 6))
    for j, anc in enumerate(ANCESTOR_INDEX):
        Wj = ktd_w[j]
        G[j] = Wj[:, :HID]
        off = HID
        for i in anc:
            A = Wj[:, off:off + 6]; off += 6
            G[j] += A @ G[i]
            H[j] += A @ H[i]
            c[j] += A @ c[i]
        # reference concatenates init_pose[..., j:j+6] (overlapping slice)
        H[j][:, j:j + 6] += Wj[:, off:off + 6]
        c[j] += ktd_b[j]
    G = G.reshape(144, HID); H = H.reshape(144, 144); c = c.reshape(144)

    Dp, Ds, Dc = deccam_w[:, :HID], deccam_w[:, HID:2 * HID], deccam_w[:, 2 * HID:]

    W = np.zeros((NOUT, 413)); b = np.zeros(NOUT)
    W[0:3, 0:256] = Dp @ F2x + Ds @ F1x
    W[0:3, 256:400] = Dp @ F2p
    W[0:3, 400:410] = Ds @ F1s
    W[0:3, 410:413] = Dc + np.eye(3)
    b[0:3] = Dp @ fc2_b + Ds @ fc1_b + deccam_b

    W[3:147, 0:256] = G @ F2x
    W[3:147, 256:400] = G @ F2p + H + np.eye(144)
    b[3:147] = G @ fc2_b + c

    W[147:157, 0:256] = decshape_w @ F1x
    W[147:157, 400:410] = decshape_w @ F1s + np.eye(10)
    b[147:157] = decshape_w @ fc1_b + decshape_b
    return W.astype(np.float32), b.astype(np.float32)


def _build_program():
    import concourse.bass as bass
    import concourse.tile as tile
    from concourse import bacc, mybir

    f32 = mybir.dt.float32
    f32r = mybir.dt.float32r
    nc = bacc.Bacc("TRN2", target_bir_lowering=False, debug=False,
                   num_devices=NCORES)
    # activations, feature-major: chunks 0..2 packed [128, 3, TPC], chunk 3 [30, TPC]
    # float32r end-to-end: same 4-byte data, PE streams 1 cycle/row vs 4 for f32
    vt012 = nc.declare_dram_parameter("vt012", [128, 3, TPC], f32r, isOutput=False)
    vt3 = nc.declare_dram_parameter("vt3", [30, TPC], f32r, isOutput=False)
    # weights packed [128, 4, NOUT]; chunk 3 rows 30..127 are zero (unused)
    wt = nc.declare_dram_parameter("wt", [128, 4, NOUT], f32r, isOutput=False)
    ot = nc.declare_dram_parameter("ot", [NOUT, TPC], f32, isOutput=True)

    with tile.TileContext(nc) as tc:
        with (
            tc.tile_pool(name="wpool", bufs=1) as wpool,
            tc.tile_pool(name="rhs", bufs=4) as rpool,
            tc.tile_pool(name="outp", bufs=3) as opool,
            tc.tile_pool(name="psum", bufs=4, space=bass.MemorySpace.PSUM) as ppool,
        ):
            w = wpool.tile([128, 4, NOUT], f32r, tag="w", name="w")
            nc.scalar.dma_start(w[:], wt[:])

            for t in range(NT):
                tok = bass.ts(t, TW)
                # alternate the two HWDGE rings (sync=qSP, scalar=qAct) so
                # input streams run on both rings concurrently
                ring = nc.sync if t % 2 == 0 else nc.scalar
                r012 = rpool.tile([128, 3, TW], f32r, tag="r012", name=f"r012_{t}")
                ring.dma_start(r012[:], vt012[:, :, tok])
                r3 = rpool.tile([30, TW], f32r, tag="r3", name=f"r3_{t}")
                ring.dma_start(r3[:], vt3[:, tok])

                otiles = []
                for mi, (m0, dm) in enumerate(MCH):
                    o = opool.tile([dm, TW], f32, tag=f"o{mi}", name=f"o{mi}_{t}")
                    for h in range(TW // 512):
                        hs = bass.ts(h, 512)
                        ps = ppool.tile([dm, 512], f32, tag=f"ps{mi}",
                                        name=f"ps{mi}_{t}_{h}")
                        for k in range(4):
                            if k < 3:
                                lhsT, rhs = w[:, k, m0:m0 + dm], r012[:, k, hs]
                            else:
                                lhsT, rhs = w[0:30, 3, m0:m0 + dm], r3[:, hs]
                            # float32r streams at 1 cycle/row for N>=256
                            # (plain fp32 pays 4x); same 4-byte data
                            nc.tensor.matmul(ps[:], lhsT, rhs,
                                             start=(k == 0), stop=(k == 3))
                        nc.vector.tensor_copy(o[:, hs], ps[:])
                    otiles.append((m0, dm, o))

                # stores go out on gpsimd's SWDGE queues, leaving both
                # HWDGE rings free for the input streams
                for m0, dm, o in otiles:
                    nc.gpsimd.dma_start(ot[m0:m0 + dm, tok], o[:])
    nc.compile()
    return nc


def _get_program():
    if "nc" not in _PROG:
        _PROG["nc"] = _build_program()
    return _PROG["nc"]


def _make_in_maps(x, init_pose, init_shape, init_cam, fc1_w, fc1_b, fc2_w,
                  fc2_b, decshape_w, decshape_b, deccam_w, deccam_b, ktd_w,
                  ktd_b):
    x = np.asarray(x, dtype=np.float32)
    init_pose = np.asarray(init_pose, dtype=np.float32)
    init_shape = np.asarray(init_shape, dtype=np.float32)
    init_cam = np.asarray(init_cam, dtype=np.float32)

    W, b = _compose_affine(
        np.asarray(fc1_w), np.asarray(fc1_b), np.asarray(fc2_w),
        np.asarray(fc2_b), np.asarray(decshape_w), np.asarray(decshape_b),
        np.asarray(deccam_w), np.asarray(deccam_b), np.asarray(ktd_w),
        np.asarray(ktd_b))
    # augment with bias column; device weight layout is [128, 4, 157]
    # (partition p, k-chunk, out-feature), chunk 3 zero-padded past row 30
    W_aug = np.concatenate([W, b[:, None]], axis=1)        # [157, 414]
    wtk = W_aug.T                                           # [414, 157]
    wt = np.zeros((4, 128, NOUT), np.float32)
    wt[0:3] = wtk[0:384].reshape(3, 128, NOUT)
    wt[3, 0:30] = wtk[384:414]
    wt = np.ascontiguousarray(wt.transpose(1, 0, 2))        # [128, 4, 157]

    xs = x.reshape(NCORES, TPC, 256)
    ps = init_pose.reshape(NCORES, TPC, 144)
    ss = init_shape.reshape(NCORES, TPC, 10)
    cs = init_cam.reshape(NCORES, TPC, 3)

    in_maps = []
    for i in range(NCORES):
        v = np.empty((KV, TPC), np.float32)                 # feature-major shard
        v[0:256] = xs[i].T
        v[256:400] = ps[i].T
        v[400:410] = ss[i].T
        v[410:413] = cs[i].T
        v[413] = 1.0
        in_maps.append({
            "vt012": np.ascontiguousarray(
                v[0:384].reshape(3, 128, TPC).transpose(1, 0, 2)),
            "vt3": np.ascontiguousarray(v[384:414]),
            "wt": wt,
        })
    return in_maps


def _assemble(results):
    out_t = np.empty((NOUT, NTOK), np.float32)
    for i in range(NCORES):
        out_t[:, i * TPC:(i + 1) * TPC] = results[i]["ot"]
    return np.ascontiguousarray(out_t.T)


def kernel(x, init_pose, init_shape, init_cam, fc1_w, fc1_b, fc2_w, fc2_b,
           decshape_w, decshape_b, deccam_w, deccam_b, ktd_w, ktd_b):
    from concourse.bass_utils import run_bass_kernel_spmd

    in_maps = _make_in_maps(x, init_pose, init_shape, init_cam, fc1_w, fc1_b,
                            fc2_w, fc2_b, decshape_w, decshape_b, deccam_w,
                            deccam_b, ktd_w, ktd_b)
    nc = _get_program()
    res = run_bass_kernel_spmd(nc, in_maps, list(range(NCORES)))
    return _assemble(res.results)



# revision 2
# speedup vs baseline: 1.4404x; 1.4404x over previous
"""Trainium2 kernel for nn_HSCR_67396626809127 (gnn_message_passing).

The reference network (fc1/fc2 -> 24-step KTD kinematic-tree recurrence ->
cam/pose/shape heads) contains no nonlinearity (dropout is identity in eval
mode), so the whole module is one affine map:

    out[157] = W @ [x(256) | init_pose(144) | init_shape(10) | init_cam(3)] + b

W [157,413] / b [157] are composed on host in float64 from the small weight
tensors (<5MB total), with the bias folded in as a constant-ones feature row
(K = 414).  The device runs a single data-parallel matmul over the
B*T = 32768 tokens: each of the 8 cores handles 4096 tokens.

Everything on-device is bf16 (rel-err gate is 2e-2; bf16 costs ~1e-3):
  - halves HBM traffic vs f32 (9.7 MB -> 4.9 MB per core, ~358 GB/s/core)
  - PE streams bf16 at 1 col/cycle (f32r pays 2) and FWL kicks in
Inputs are packed feature-major on host so every DMA is 128-partition with
multi-KB contiguous runs per partition; all bulk DMA goes on the two HWDGE
rings (sync + scalar), which together sustain ~350 GB/s.  A burst of dummy
matmuls on the weight tile warms the PE HAM clock gate (1.2 -> 2.4 GHz)
while the first input tile streams in.
"""

import numpy as np
import ml_dtypes

BF16 = ml_dtypes.bfloat16

ANCESTOR_INDEX = [[], [0], [0], [0], [0, 1], [0, 2], [0, 3], [0, 1, 4],
                  [0, 2, 5], [0, 3, 6], [0, 1, 4, 7], [0, 2, 5, 8],
                  [0, 3, 6, 9], [0, 3, 6, 9], [0, 3, 6, 9], [0, 3, 6, 9, 12],
                  [0, 3, 6, 9, 13], [0, 3, 6, 9, 14], [0, 3, 6, 9, 13, 16],
                  [0, 3, 6, 9, 14, 17], [0, 3, 6, 9, 13, 16, 18],
                  [0, 3, 6, 9, 14, 17, 19], [0, 3, 6, 9, 13, 16, 18, 20],
                  [0, 3, 6, 9, 14, 17, 19, 21]]
HID = 1024
NCORES = 8
B, T = 2048, 16
NTOK = B * T                 # 32768
TPC = NTOK // NCORES         # 4096 tokens per core
NOUT = 157                   # [cam 3 | pose 144 | shape 10]
KV = 414                     # 413 input features + ones row (bias)
TW = 1024                    # tokens per SBUF tile
NT = TPC // TW               # 4 tiles per core
NWARM = 12                   # PE warm-up matmuls (HAM clock gate)

_PROG = {}


def _compose_affine(fc1_w, fc1_b, fc2_w, fc2_b, decshape_w, decshape_b,
                    deccam_w, deccam_b, ktd_w, ktd_b):
    """Fold the whole network into out = v @ W.T + b, v = [x|pose|shape|cam]."""
    f8 = np.float64
    fc1_w, fc1_b = fc1_w.astype(f8), fc1_b.astype(f8)
    fc2_w, fc2_b = fc2_w.astype(f8), fc2_b.astype(f8)
    decshape_w, decshape_b = decshape_w.astype(f8), decshape_b.astype(f8)
    deccam_w, deccam_b = deccam_w.astype(f8), deccam_b.astype(f8)
    ktd_w, ktd_b = ktd_w.astype(f8), ktd_b.astype(f8)

    F1x, F1s = fc1_w[:, :256], fc1_w[:, 256:266]
    F2x, F2p = fc2_w[:, :256], fc2_w[:, 256:400]

    # KTD recurrence -> pose_out = G @ xc_pose + H @ init_pose + c
    G = np.zeros((24, 6, HID)); H = np.zeros((24, 6, 144)); c = np.zeros((24, 6))
    for j, anc in enumerate(ANCESTOR_INDEX):
        Wj = ktd_w[j]
        G[j] = Wj[:, :HID]
        off = HID
        for i in anc:
            A = Wj[:, off:off + 6]; off += 6
            G[j] += A @ G[i]
            H[j] += A @ H[i]
            c[j] += A @ c[i]
        # reference concatenates init_pose[..., j:j+6] (overlapping slice)
        H[j][:, j:j + 6] += Wj[:, off:off + 6]
        c[j] += ktd_b[j]
    G = G.reshape(144, HID); H = H.reshape(144, 144); c = c.reshape(144)

    Dp, Ds, Dc = deccam_w[:, :HID], deccam_w[:, HID:2 * HID], deccam_w[:, 2 * HID:]

    W = np.zeros((NOUT, 413)); b = np.zeros(NOUT)
    W[0:3, 0:256] = Dp @ F2x + Ds @ F1x
    W[0:3, 256:400] = Dp @ F2p
    W[0:3, 400:410] = Ds @ F1s
    W[0:3, 410:413] = Dc + np.eye(3)
    b[0:3] = Dp @ fc2_b + Ds @ fc1_b + deccam_b

    W[3:147, 0:256] = G @ F2x
    W[3:147, 256:400] = G @ F2p + H + np.eye(144)
    b[3:147] = G @ fc2_b + c

    W[147:157, 0:256] = decshape_w @ F1x
    W[147:157, 400:410] = decshape_w @ F1s + np.eye(10)
    b[147:157] = decshape_w @ fc1_b + decshape_b
    return W.astype(np.float32), b.astype(np.float32)


def _build_program():
    import concourse.bass as bass
    import concourse.tile as tile
    from concourse import bacc, mybir

    f32 = mybir.dt.float32
    bf16 = mybir.dt.bfloat16
    nc = bacc.Bacc("TRN2", target_bir_lowering=False, debug=False,
                   num_devices=NCORES)
    # feature-major activations: chunks 0..2 (features 0..383) packed per
    # token-tile so each tile DMA is one contiguous 6KB run per partition
    vin = nc.declare_dram_parameter("vin", [128, NT, 3, TW], bf16,
                                    isOutput=False)
    # chunk 3 (features 384..413): whole-core in one upfront DMA
    vt3 = nc.declare_dram_parameter("vt3", [30, TPC], bf16, isOutput=False)
    # weights packed [128, 4, NOUT]; chunk 3 rows 30..127 unused
    wt = nc.declare_dram_parameter("wt", [128, 4, NOUT], bf16, isOutput=False)
    # outputs: rows 0..127 and 128..156, tile-major for contiguous stores
    o0 = nc.declare_dram_parameter("o0", [128, NT, TW], bf16, isOutput=True)
    o1 = nc.declare_dram_parameter("o1", [29, NT, TW], bf16, isOutput=True)

    with tile.TileContext(nc) as tc:
        with (
            tc.tile_pool(name="wpool", bufs=1) as wpool,
            tc.tile_pool(name="rhs", bufs=NT) as rpool,
            tc.tile_pool(name="outp", bufs=2) as opool,
            tc.tile_pool(name="psum", bufs=2, space=bass.MemorySpace.PSUM) as ppool,
            tc.tile_pool(name="warm", bufs=1, space=bass.MemorySpace.PSUM) as wmpool,
        ):
            w = wpool.tile([128, 4, NOUT], bf16, tag="w", name="w")
            nc.sync.dma_start(w[:], wt[:])
            v3 = wpool.tile([30, TPC], bf16, tag="v3", name="v3")
            nc.scalar.dma_start(v3[:], vt3[:])

            # queue all input-tile loads upfront, alternating the two
            # HWDGE rings (sync=qSP, scalar=qAct) so both stream in parallel
            rtiles = []
            for t in range(NT):
                ring = nc.sync if t % 2 == 0 else nc.scalar
                r = rpool.tile([128, 3, TW], bf16, tag="r", name=f"r_{t}")
                ring.dma_start(r[:], vin[:, t])
                rtiles.append(r)

            # dummy matmuls on the weight tile warm the PE HAM clock gate
            # (K=4/8 -> 8/8) while the first input tile is still in flight
            for i in range(NWARM):
                pw = wmpool.tile([128, 314], f32, tag="wm", name=f"wm_{i}")
                nc.tensor.matmul(pw[:], w[:, 0, 0:128], w[:, 0:2, :],
                                 start=True, stop=True)

            for t in range(NT):
                r = rtiles[t]
                ot0 = opool.tile([128, TW], bf16, tag="o0", name=f"o0_{t}")
                ot1 = opool.tile([29, TW], bf16, tag="o1", name=f"o1_{t}")
                for h in range(2):
                    hs = bass.ts(h, 512)
                    cs = bass.ts(2 * t + h, 512)  # column slice within v3
                    ps0 = ppool.tile([128, 512], f32, tag="ps0",
                                     name=f"ps0_{t}_{h}")
                    for k in range(4):
                        if k < 3:
                            lhsT, rhs = w[:, k, 0:128], r[:, k, hs]
                        else:
                            lhsT, rhs = w[0:30, 3, 0:128], v3[:, cs]
                        nc.tensor.matmul(ps0[:], lhsT, rhs,
                                         start=(k == 0), stop=(k == 3))
                    ps1 = ppool.tile([29, 512], f32, tag="ps1",
                                     name=f"ps1_{t}_{h}")
                    for k in range(4):
                        if k < 3:
                            lhsT, rhs = w[:, k, 128:157], r[:, k, hs]
                        else:
                            lhsT, rhs = w[0:30, 3, 128:157], v3[:, cs]
                        nc.tensor.matmul(ps1[:], lhsT, rhs,
                                         start=(k == 0), stop=(k == 3))
                    # cast f32 PSUM -> bf16 SBUF; big chunk on DVE, small on ACT
                    nc.vector.tensor_copy(ot0[:, hs], ps0[:])
                    nc.scalar.copy(ot1[:, hs], ps1[:])

                # stores ride the opposite HWDGE ring from this tile's load
                oring = nc.scalar if t % 2 == 0 else nc.sync
                oring.dma_start(o0[:, t], ot0[:])
                oring.dma_start(o1[:, t], ot1[:])
    nc.compile()
    return nc


def _get_program():
    if "nc" not in _PROG:
        _PROG["nc"] = _build_program()
    return _PROG["nc"]


def _make_in_maps(x, init_pose, init_shape, init_cam, fc1_w, fc1_b, fc2_w,
                  fc2_b, decshape_w, decshape_b, deccam_w, deccam_b, ktd_w,
                  ktd_b):
    x = np.asarray(x, dtype=np.float32)
    init_pose = np.asarray(init_pose, dtype=np.float32)
    init_shape = np.asarray(init_shape, dtype=np.float32)
    init_cam = np.asarray(init_cam, dtype=np.float32)

    W, b = _compose_affine(
        np.asarray(fc1_w), np.asarray(fc1_b), np.asarray(fc2_w),
        np.asarray(fc2_b), np.asarray(decshape_w), np.asarray(decshape_b),
        np.asarray(deccam_w), np.asarray(deccam_b), np.asarray(ktd_w),
        np.asarray(ktd_b))
    # augment with bias column; device weight layout is [128, 4, 157]
    # (partition p, k-chunk, out-feature), chunk 3 zero-padded past row 30
    W_aug = np.concatenate([W, b[:, None]], axis=1)        # [157, 414]
    wtk = W_aug.T.astype(BF16)                              # [414, 157]
    wt = np.zeros((4, 128, NOUT), BF16)
    wt[0:3] = wtk[0:384].reshape(3, 128, NOUT)
    wt[3, 0:30] = wtk[384:414]
    wt = np.ascontiguousarray(wt.transpose(1, 0, 2))        # [128, 4, 157]

    xs = x.reshape(NCORES, TPC, 256)
    ps = init_pose.reshape(NCORES, TPC, 144)
    ss = init_shape.reshape(NCORES, TPC, 10)
    cs = init_cam.reshape(NCORES, TPC, 3)

    in_maps = []
    for i in range(NCORES):
        v = np.empty((KV, TPC), np.float32)                 # feature-major shard
        v[0:256] = xs[i].T
        v[256:400] = ps[i].T
        v[400:410] = ss[i].T
        v[410:413] = cs[i].T
        v[413] = 1.0
        vb = v.astype(BF16)
        # vin[p, t, c, w] = v[c*128+p, t*TW+w]
        vin = np.ascontiguousarray(
            vb[0:384].reshape(3, 128, NT, TW).transpose(1, 2, 0, 3))
        in_maps.append({
            "vin": vin,
            "vt3": np.ascontiguousarray(vb[384:414]),
            "wt": wt,
        })
    return in_maps


def _assemble(results):
    out_t = np.empty((NOUT, NTOK), np.float32)
    for i in range(NCORES):
        sl = slice(i * TPC, (i + 1) * TPC)
        out_t[0:128, sl] = results[i]["o0"].reshape(128, TPC)
        out_t[128:157, sl] = results[i]["o1"].reshape(29, TPC)
    return np.ascontiguousarray(out_t.T)


def kernel(x, init_pose, init_shape, init_cam, fc1_w, fc1_b, fc2_w, fc2_b,
           decshape_w, decshape_b, deccam_w, deccam_b, ktd_w, ktd_b):
    from concourse.bass_utils import run_bass_kernel_spmd

    in_maps = _make_in_maps(x, init_pose, init_shape, init_cam, fc1_w, fc1_b,
                            fc2_w, fc2_b, decshape_w, decshape_b, deccam_w,
                            deccam_b, ktd_w, ktd_b)
    nc = _get_program()
    res = run_bass_kernel_spmd(nc, in_maps, list(range(NCORES)))
    return _assemble(res.results)
